# revision 23
# baseline (speedup 1.0000x reference)
"""CASVDDenseMul fused kernel for 8 Trainium2 NeuronCores.

Reference computation (fp32):
    chi = sigmoid(context @ W + B)          # [B, R]
    t   = (inputs @ U) * (S * chi)          # [B, R]
    out = relu(t @ V.T + 2*bias)            # [B, UNITS]

Sharding: data-parallel over batch; each of the 8 cores handles 512 rows.
All factor weights (U, S, V, W, B, bias) are replicated.

Design notes (v6 -- single-queue total order, measured-latency driven):
  - All-bf16 transport (fp8 anywhere measured over the 2e-2 gate).
    13.4MB/core HBM traffic; PE work ~29us; wire ~34us => wire-bound.
  - ONE HWDGE ring (sync) carries EVERY transfer in consumption order.
    Measured: a single queue sustains 390-418 GB/s solo -- same as two
    queues combined -- while two queues round-robin at packet
    granularity, which let early output waves steal bandwidth from the
    input tail and let issue-side skew reorder arrivals.  Single-queue
    FIFO gives a total order on the wire by construction: inputs
    [U/x interleaved, wctx, VT], then the 16 output waves.
  - Measured: each DMA->compute dependency pays ~2.5us completion
    latency (HBM write-receipt before the semaphore fires) on top of
    wire time.  Consequences baked in here: (a) no batch sub-blocking
    -- with the PE saturated from t' onward, end = t'_time + remaining
    PE work, and moving any mm1 after t' costs (PE time) more than the
    earlier t' saves (wire time); (b) x streams in 0.52MB pieces and
    vt piece 0 in halves, so the last dependency's receipt overlaps
    the next transfer's wire time; (c) t' is computed in 8 [P,128]
    column blocks emitted in wave-consumption order, so mm2's first
    wave starts ~0.2us after mm1's last accumulation, not ~1.7us.
  - chi runs entirely off the critical path: its two matmul groups get
    their OWN psum banks (from the wave pool, which they recycle into)
    so the groups+sigmoids pipeline, and the scalar engine issues no
    DMAs, so sigmoids can't be trapped behind DMA-issue semaphore
    recycling (both failure modes observed in earlier revisions).
  - mm2 waves: two single-bank [128,512] psum tiles per 1024-unit wave
    from a 6-deep pool; 512-wide matmuls, rank pairs share one
    LDWEIGHTS, evacuation split ACT(h0)/DVE(h1); outputs stage in SBUF
    (16 dedicated buffers) and drain the single ring FIFO behind the
    inputs.
  - PSUM: psum_t [P,1024] fp32 = 2 aligned banks (rt-major), wave pool
    6 banks => 8 total.  start=True clears has_written BANK-wide, so
    only the first matmul into a shared bank carries it.
"""

import numpy as np
import ml_dtypes

from concourse import bacc, mybir
from concourse import tile
from concourse.bass_utils import run_bass_kernel_spmd

N_CORES = 8
B_SZ, N_IN, N_CTX, UNITS, RANK = 4096, 4096, 512, 4096, 256
BS = B_SZ // N_CORES   # 512 batch rows per core

P = 128
KC_IN = N_IN // P      # 32 contraction chunks for x @ U
KC_CTX = N_CTX // P    # 4  contraction chunks for ctx @ W
RT = RANK // P         # 2  rank tiles
NU = 4                 # U stream pieces
KPU = KC_IN // NU      # 8 chunks per U piece
# x stream piece sizes in chunks: big pieces early (small transfers ramp
# the SDMA pipeline slowly -- measured 197-325GB/s vs ~400 steady), then
# a fine taper so the PE can chew the last chunks right behind the last
# completion semaphores.
X_PIECES = [8, 8, 4, 4, 2, 2, 2, 2]
NBT = BS // P          # 4 batch tiles
NW = 4                 # VT pieces / unit-wave groups (1024 units each)
WU = UNITS // NW       # 1024 units per wave

BF16 = mybir.dt.bfloat16
FP32 = mybir.dt.float32
FP32R = mybir.dt.float32r

bf16 = ml_dtypes.bfloat16


def _build_nc(use_b, use_bias):
    nc = bacc.Bacc("TRN2", target_bir_lowering=False, debug=False, enable_asserts=False)

    wctx = nc.declare_dram_parameter("wctx", [P, KC_CTX * (RANK + BS)], BF16, isOutput=False)
    u8 = nc.declare_dram_parameter("u8", [NU, P, KPU * RANK], BF16, isOutput=False)
    xg = nc.declare_dram_parameter("xg", [P, KC_IN * BS], BF16, isOutput=False)
    vt4 = nc.declare_dram_parameter("vt4", [NW, P, RT * WU], BF16, isOutput=False)
    if use_b:
        bvec = nc.declare_dram_parameter("bvec", [P, RT], FP32, isOutput=False)
    if use_bias:
        brow = nc.declare_dram_parameter("brow", [1, P + UNITS], FP32R, isOutput=False)
    out_d = nc.declare_dram_parameter("out_d", [BS, UNITS], BF16, isOutput=True)

    with tile.TileContext(nc) as tc:
        with (
            tc.tile_pool(name="small", bufs=1) as small,
            tc.tile_pool(name="stream", bufs=1) as stream,
            tc.tile_pool(name="acts", bufs=1) as acts,
            tc.tile_pool(name="ostage", bufs=16) as ostage,
            tc.tile_pool(name="pt", bufs=1, space="PSUM") as pt,
            tc.tile_pool(name="pout", bufs=6, space="PSUM") as pout,
        ):
            # ---- SBUF tiles ----
            wctx_sb = small.tile([P, KC_CTX * (RANK + BS)], BF16, tag="wctx")
            u_sb = small.tile([P, NU, KPU * RANK], BF16, tag="u")
            # One contiguous x tile, chunk-major; pieces are column
            # slices (Tile's dependency tracking is AP-range-granular,
            # so readers wait only on the slice that feeds them).
            x_sb = stream.tile([P, KC_IN * BS], BF16, tag="x")
            vt_sb = small.tile([P, NW, RT * WU], BF16, tag="vt")
            if use_b:
                bvec_sb = small.tile([P, RT], FP32, tag="bvec")
            if use_bias:
                brow_sb = small.tile([1, P + UNITS], FP32R, tag="brow")
            s_chi = acts.tile([P, RT, BS], FP32, tag="schi")
            t_sb = acts.tile([P, RT, BS], BF16, tag="tsb")
            junk = acts.tile([P, P], BF16, tag="junk")

            # ---- DMA issue queue: ONE ring, consumption order.
            # u/x interleave for mm1 in 0.26MB x pieces; wctx mid-stream
            # (chi is a gap-filler); vt0 BEFORE the x tail so its
            # completion receipt retires off the critical path and mm2's
            # first wave is gated only by t'; vt1-3 last (they arrive
            # ahead of the PE's wave consumption); outputs (emitted
            # inside the wave bodies below) drain FIFO after everything.
            xb = []   # chunk boundaries of the x pieces
            acc = 0
            for nch in X_PIECES:
                xb.append((acc, acc + nch))
                acc += nch

            def dma_x(i):
                k0, k1 = xb[i]
                nc.sync.dma_start(x_sb[:, k0 * BS: k1 * BS], xg[:, k0 * BS: k1 * BS])

            nc.sync.dma_start(u_sb[:, 0, :], u8[0])
            dma_x(0)                                   # chunks 0-7
            if use_b:
                nc.sync.dma_start(bvec_sb[:], bvec[:])
            if use_bias:
                nc.sync.dma_start(brow_sb[:], brow[:])
            nc.sync.dma_start(u_sb[:, 1, :], u8[1])
            dma_x(1)                                   # chunks 8-15
            nc.sync.dma_start(wctx_sb[:], wctx[:])
            nc.sync.dma_start(vt_sb[:, 0, :], vt4[0])
            nc.sync.dma_start(u_sb[:, 2, :], u8[2])
            dma_x(2)                                   # chunks 16-19
            dma_x(3)                                   # chunks 20-23
            nc.sync.dma_start(u_sb[:, 3, :], u8[3])
            for i in range(4, len(X_PIECES)):
                dma_x(i)                               # chunks 24-31, fine
            for w in range(1, NW):
                nc.sync.dma_start(vt_sb[:, w, :], vt4[w])

            psum_t = pt.tile([P, RT * BS], FP32, tag="pt")

            # ---- PE warm-up: keep the HAM activity window busy from t=0
            # so the clock gate lifts to 2.4 GHz before the real stream.
            # Targets psum_t, whose contents mm1's start=True clears.
            nc.gpsimd.memset(junk[:], 0.0)
            for _ in range(16):
                nc.tensor.matmul(
                    psum_t[:, :P], junk[:], junk[:],
                    start=True, stop=True, skip_group_check=True,
                )

            # ---- chi' = sigmoid(W.T @ ctxT + B)  (S folded into U) ----
            # chi psum tiles come from the wave pool (same [P,512] fp32
            # shape): each matmul group gets its OWN bank, so the groups
            # and sigmoids pipeline instead of serializing; the banks
            # recycle into the wave rotation afterwards.
            for rt in range(RT):
                psum_chi = pout.tile([P, BS], FP32, tag="po", name=f"pchi{rt}")
                for k in range(KC_CTX):
                    base = k * (RANK + BS)
                    nc.tensor.matmul(
                        psum_chi[:],
                        wctx_sb[:, base + rt * P: base + (rt + 1) * P],
                        wctx_sb[:, base + RANK: base + RANK + BS],
                        start=(k == 0), stop=(k == KC_CTX - 1),
                        skip_group_check=True,
                    )
                nc.scalar.activation(
                    s_chi[:, rt, :], psum_chi[:],
                    mybir.ActivationFunctionType.Sigmoid,
                    bias=(bvec_sb[:, rt:rt + 1] if use_b else 0.0), scale=1.0,
                )

            # ---- mm1: psum_t[:, rt*BS:(rt+1)*BS] += U'_k.T @ x_k ----
            # The rt groups live in SEPARATE psum banks of the 2-bank
            # tile, so EACH group's k=0 matmul must carry start=True to
            # clear its own bank's has_written bits (stale from the
            # previous execution otherwise).
            for k in range(KC_IN):
                up = k // KPU             # u piece holding chunk k
                ju = k - up * KPU         # chunk index within it
                for rt in range(RT):
                    nc.tensor.matmul(
                        psum_t[:, rt * BS:(rt + 1) * BS],
                        u_sb[:, up, ju * RANK + rt * P: ju * RANK + (rt + 1) * P],
                        x_sb[:, k * BS:(k + 1) * BS],
                        start=(k == 0),
                        stop=(k == KC_IN - 1),
                        skip_group_check=True,
                    )

            # ---- t' = psum_t * chi, in [P,128] blocks emitted in wave-
            # consumption order (bt-major) so mm2's first wave needs only
            # the first two blocks, not the whole 1.7us multiply.
            # high_priority keeps the vector engine from interleaving
            # wave evacuations ahead of the later blocks, which would
            # pace mm2's bt2/bt3 waves.
            with tc.high_priority():
                for bt in range(NBT):
                    for rt in range(RT):
                        sl = slice(bt * P, (bt + 1) * P)
                        nc.vector.tensor_mul(
                            t_sb[:, rt, sl],
                            psum_t[:, rt * BS + bt * P: rt * BS + (bt + 1) * P],
                            s_chi[:, rt, sl],
                        )

            # ---- mm2 waves: out[bt-rows, wave-units] = t'.T @ VT,
            # (+ 2*bias), relu, stage to SBUF, DMA on the single ring.
            widx = 0
            for w in range(NW):
                for bt in range(NBT):
                    pw = [pout.tile([P, 512], FP32, tag="po", name=f"po{widx}h{h}")
                          for h in range(2)]
                    for rt in range(RT):
                        for h in range(2):
                            nc.tensor.matmul(
                                pw[h][:],
                                t_sb[:, rt, bt * P:(bt + 1) * P],
                                vt_sb[:, w, rt * WU + h * 512: rt * WU + (h + 1) * 512],
                                start=(rt == 0),
                                stop=(rt == RT - 1 and not use_bias),
                                skip_group_check=True,
                            )
                    if use_bias:
                        for h in range(2):
                            nc.tensor.matmul(
                                pw[h][:],
                                brow_sb[:, 0:P],
                                brow_sb[:, P + w * WU + h * 512: P + w * WU + (h + 1) * 512],
                                start=False, stop=True,
                                skip_group_check=True,
                            )
                    o_sb = ostage.tile([P, WU], BF16, tag="osb")
                    nc.scalar.activation(
                        o_sb[:, 0:512], pw[0][:],
                        mybir.ActivationFunctionType.Relu,
                    )
                    nc.vector.tensor_scalar(
                        o_sb[:, 512:WU], pw[1][:], 0.0, None,
                        op0=mybir.AluOpType.max,
                    )
                    rows = slice(bt * P, (bt + 1) * P)
                    cols = slice(w * WU, (w + 1) * WU)
                    if widx == NW * NBT - 1:
                        # Last wave: ship each half as soon as its own
                        # evacuation lands -- trims the final wire tail.
                        nc.sync.dma_start(
                            out_d[rows, w * WU: w * WU + 512], o_sb[:, 0:512])
                        nc.sync.dma_start(
                            out_d[rows, w * WU + 512:(w + 1) * WU], o_sb[:, 512:WU])
                    else:
                        nc.sync.dma_start(out_d[rows, cols], o_sb[:])
                    widx += 1

    nc.finalize()
    return nc


_NC_CACHE = {}


def _get_nc(use_b=False, use_bias=False):
    key = (use_b, use_bias)
    if key not in _NC_CACHE:
        _NC_CACHE[key] = _build_nc(use_b, use_bias)
    return _NC_CACHE[key]


def _round_fp32r(a):
    """Round fp32 to the fp32r grid (11-bit mantissa; low 12 bits zero)."""
    u = np.ascontiguousarray(a, dtype=np.float32).view(np.uint32)
    r = (u + np.uint32(0x7FF) + ((u >> np.uint32(12)) & np.uint32(1))) & np.uint32(0xFFFFF000)
    return r.view(np.float32)


def build(inputs, context, U, S, V, W, B, bias):
    """Host-side packing: returns (nc, in_maps)."""
    use_b = bool(np.any(np.asarray(B)))
    use_bias = bool(np.any(np.asarray(bias)))

    # U with S folded into its columns, chunked for the stream:
    # u8[q, p, j*RANK + r] = (U*S)[(q*KPU+j)*128 + p, r]
    US = (np.asarray(U, np.float32) * np.asarray(S, np.float32)[None, :]).astype(bf16)
    u8 = np.ascontiguousarray(
        US.reshape(NU, KPU, P, RANK).transpose(0, 2, 1, 3).reshape(NU, P, KPU * RANK)
    )

    # VT pieces: vt4[c, p, rt*WU + m'] = V.T[rt*128 + p, c*WU + m']
    VTb = np.asarray(V, np.float32).T.astype(bf16)          # [RANK, UNITS]
    vt4 = np.ascontiguousarray(
        VTb.reshape(RT, P, NW, WU).transpose(2, 1, 0, 3).reshape(NW, P, RT * WU)
    )

    Wk = np.asarray(W, np.float32).astype(bf16).reshape(KC_CTX, P, RANK)
    ctxT = np.asarray(context, np.float32).astype(bf16).T   # [N_CTX, B_SZ]
    xT = np.asarray(inputs, np.float32).astype(bf16).T      # [N_IN, B_SZ]

    bvec = np.ascontiguousarray(np.asarray(B, np.float32).reshape(RT, P).T)
    brow = np.empty((1, P + UNITS), np.float32)
    brow[0, :P] = 1.0
    brow[0, P:] = 2.0 * np.asarray(bias, np.float32)
    brow = _round_fp32r(brow)

    in_maps = []
    for c in range(N_CORES):
        sl = slice(c * BS, (c + 1) * BS)
        # wctx[p, k*(RANK+BS) + ...] = [W_k | ctx_k] per contraction chunk
        wc = np.empty((KC_CTX, P, RANK + BS), bf16)
        wc[:, :, :RANK] = Wk
        wc[:, :, RANK:] = ctxT[:, sl].reshape(KC_CTX, P, BS)
        wctx = np.ascontiguousarray(
            wc.transpose(1, 0, 2).reshape(P, KC_CTX * (RANK + BS))
        )
        # xg[p, k*BS + b] = xT[k*128 + p, c*BS + b]  (chunk-major)
        xc = xT[:, sl]                                       # [N_IN, BS]
        xgc = np.ascontiguousarray(
            xc.reshape(KC_IN, P, BS).transpose(1, 0, 2).reshape(P, KC_IN * BS)
        )
        m = {"wctx": wctx, "u8": u8, "xg": xgc, "vt4": vt4}
        if use_b:
            m["bvec"] = bvec
        if use_bias:
            m["brow"] = brow
        in_maps.append(m)
    return _get_nc(use_b, use_bias), in_maps


def gather_out(results):
    out = np.empty((B_SZ, UNITS), dtype=np.float32)
    for c in range(N_CORES):
        out[c * BS:(c + 1) * BS, :] = results[c]["out_d"].astype(np.float32)
    return out


def kernel(inputs, context, U, S, V, W, B, bias):
    nc, in_maps = build(inputs, context, U, S, V, W, B, bias)
    res = run_bass_kernel_spmd(nc, in_maps, list(range(N_CORES)))
    return gather_out(res.results)
